# revision 2
# baseline (speedup 1.0000x reference)
"""Trainium2 Bass kernel v2: multi-head attention (Graphormer-style bias+mask)
followed by a node-similarity GEMM (out = merged @ merged^T).

Sharding: pure data-parallel over batch. B=8 -> 8 NeuronCores.

Key differences vs v1:
  - S^T = K Q^T computed directly (bias supplied transposed per head from the
    host), so softmax output E^T is already in the [m-part, n-free] layout the
    A^T matmul needs -> no PE transposes.
  - mask folded into bias on the host (bf16), no identity-matmuls for mask.
  - V augmented with a ones column -> A^T matmul row 64 accumulates the
    softmax row-sums r[n] for free; normalization deferred to a single
    per-head multiply when writing mergedT (R broadcast via gpsimd).
  - final GEMM computes only the upper block-triangle (output symmetric);
    host mirrors the lower tiles.
  - batched DMAs (bias in 4-tile groups) to amortize HWDGE issue overhead.
"""

import sys

if "/opt/trn_rl_repo" not in sys.path:
    sys.path.insert(0, "/opt/trn_rl_repo")

import ml_dtypes
import numpy as np

P = 128
N = 1024
C = 512
H = 8
D = 64  # head dim
NT = N // P  # 8 row tiles
CT = C // P  # 4 channel tiles
NCORES = 8

import os

# tile indices whose bias add runs as a PE identity-matmul (rest on DVE);
# heads 1-3 carry threaded QK-projection chains so they get a lighter set
PE_ADD_I = tuple(int(c) for c in os.environ.get("KV2_PE_ADD_I", "01467"))
PE_ADD_I_QKH = tuple(int(c) for c in os.environ.get("KV2_PE_ADD_I_QKH", "01467"))
WARM_MMS = int(os.environ.get("KV2_WARM_MMS", "6"))

_CACHE = {}


def _build_nc():
    import concourse.mybir as mybir
    import concourse.tile as tile
    from concourse import bacc
    from concourse.masks import make_identity

    f32 = mybir.dt.float32
    f32r = mybir.dt.float32r
    bf16 = mybir.dt.bfloat16
    Act = mybir.ActivationFunctionType
    Alu = mybir.AluOpType

    nc = bacc.Bacc("TRN2", target_bir_lowering=False, debug=False)

    # ---- DRAM parameters (per-core) ----
    xT_d = nc.dram_tensor("xT", [C, N], bf16, kind="ExternalInput")
    wqT_d = nc.dram_tensor("wqT", [C, C], bf16, kind="ExternalInput")
    wkT_d = nc.dram_tensor("wkT", [C, C], bf16, kind="ExternalInput")
    wvT_d = nc.dram_tensor("wvT", [C, C], bf16, kind="ExternalInput")
    bqk_d = nc.dram_tensor("bqk", [P, 2 * CT], f32, kind="ExternalInput")
    bv_d = nc.dram_tensor("bv", [1, C], f32, kind="ExternalInput")
    # biasT[h, m, n] = bias[h, n, m] + (mask[n, m]-1)*2^30, bf16
    biasT_d = nc.dram_tensor("biasT", [H, N, N], bf16, kind="ExternalInput")
    out_d = nc.dram_tensor("out", [N, N], bf16, kind="ExternalOutput")

    with tile.TileContext(nc) as tc:
        with (
            tc.tile_pool(name="const", bufs=1) as constp,
            tc.tile_pool(name="pers", bufs=1) as pers,
            tc.tile_pool(name="stream", bufs=1) as stream,
            tc.tile_pool(name="psS", bufs=3, space="PSUM") as psS,
            tc.tile_pool(name="psA", bufs=2, space="PSUM") as psA,
        ):
            ident = constp.tile([P, P], f32)
            make_identity(nc, ident[:])
            ident_b = constp.tile([P, P], bf16)
            nc.vector.tensor_copy(ident_b[:], ident[:])
            # ones via activation: 0*x + 1 (walrus rejects value-memsets)
            ones_row = constp.tile([1, P], f32r)
            nc.scalar.activation(
                ones_row[:], ident[0:1, :], Act.Identity, bias=1.0, scale=0.0
            )

            warm = constp.tile([P, 1], f32)
            nc.scalar.activation(warm[:], ident[:, 0:1], Act.Exp, scale=1.0)

            # Spin the PE on dummy matmuls while the input DMAs are in
            # flight: the cost model's p-state ramp needs ~3us of continuous
            # execution before matmuls run at full clock, and the PE would
            # otherwise enter the first real chain cold.
            warm_ps = psA.tile([P, C], f32, tag="A", name="warm_ps")
            for _ in range(WARM_MMS):
                nc.tensor.matmul(
                    warm_ps[:, 0:P], ident_b[:], ident_b[:], start=True, stop=True
                )

            # ---- persistent SBUF tensors ----
            QT = [pers.tile([P, N], f32r, name=f"QT{i}") for i in range(CT)]
            KT = [pers.tile([P, N], f32r, name=f"KT{i}") for i in range(CT)]
            # V_aug[mt]: [m-part, head, 65]; col 64 of each head = 1.0
            Vg = [pers.tile([P, H, D + 1], bf16, name=f"Vg{i}") for i in range(NT)]
            # double-buffered by head parity: head h+1's exps must not clobber
            # the tiles head h's spilled A-chain matmuls still read
            ET2 = [
                [pers.tile([P, N], bf16, name=f"ET{s}_{i}") for i in range(NT)]
                for s in range(2)
            ]
            mergedT = [pers.tile([P, N], f32r, name=f"mergedT{i}") for i in range(CT)]
            bqk_sb = pers.tile([P, 2 * CT], f32, name="bqk_sb")
            bv_sb = pers.tile([1, C], f32r, name="bv_sb")
            R_sb = pers.tile([D, N], f32, name="R_sb")
            rc_sb = pers.tile([1, N], f32, name="rc_sb")

            # ---- phase 0: load x^T and W^T, compute Q^T[0], K^T[0] ----
            # The V chains and the remaining Q^T/K^T c-tiles are threaded into
            # the free pipeline slots of heads 0-3 below.
            qkvp = pers
            xTt = qkvp.tile([P, CT, N], bf16, name="xT")
            wqT = qkvp.tile([P, CT, C], bf16, name="wq")
            wkT = qkvp.tile([P, CT, C], bf16, name="wk")
            wvT = qkvp.tile([P, CT, C], bf16, name="wv")

            def load_w_cols(t, dram, c0, c1):
                nc.sync.dma_start(
                    out=t[:, :, c0:c1],
                    in_=dram[:, c0:c1].rearrange("(t p) c -> p t c", p=P),
                )

            # DMA order tuned so the first QK chains + head-0 tiles start
            # as early as possible: bqk, xT g0, wq ct0, xT g1, wk ct0, then
            # bias(0,0) [emitted at head-0 top], wv+bv [head-0 slot 0],
            # remaining wq/wk columns [head-0 slots 5-6]
            nc.sync.dma_start(out=bqk_sb[:], in_=bqk_d[:])
            nc.sync.dma_start(
                out=xTt[:, 0:2, :],
                in_=xT_d[0 : 2 * P, :].rearrange("(t p) n -> p t n", p=P),
            )
            load_w_cols(wqT, wqT_d, 0, P)
            nc.sync.dma_start(
                out=xTt[:, 2:4, :],
                in_=xT_d[2 * P : 4 * P, :].rearrange("(t p) n -> p t n", p=P),
            )
            load_w_cols(wkT, wkT_d, 0, P)

            def qk_chain(ct, which):
                # one c-tile of the Q^T (which=0) / K^T (which=1) projection;
                # copy+bias alternates Act/DVE to spread phase-1 load
                w_sb, dst = ((wqT, QT), (wkT, KT))[which]
                ps = psS.tile([P, N], f32, tag="S")
                for j in range(2):
                    for kt in range(CT):
                        nc.tensor.matmul(
                            ps[:, j * C : (j + 1) * C],
                            w_sb[:, kt, ct * P : (ct + 1) * P],
                            xTt[:, kt, j * C : (j + 1) * C],
                            start=(kt == 0),
                            stop=(kt == CT - 1),
                        )
                b = bqk_sb[:, which * CT + ct : which * CT + ct + 1]
                if which == 0 or ct == 0:
                    nc.scalar.activation(dst[ct][:], ps[:], Act.Identity, bias=b)
                else:
                    nc.vector.tensor_scalar_add(dst[ct][:], ps[:], b)

            def v_chain(mt):
                # V row-tile in the augmented [head, 65] layout (col 64 = 1);
                # uses the A psum pool (1-bank tiles), copy alternates engines
                ps = psA.tile([P, C], f32, tag="A", name=f"Vps{mt}")
                for kt in range(CT):
                    nc.tensor.matmul(
                        ps[:],
                        xTt[:, kt, mt * P : (mt + 1) * P],
                        wvT[:, kt, :],
                        start=(kt == 0),
                        stop=False,
                    )
                nc.tensor.matmul(ps[:], ones_row[:], bv_sb[:], start=False, stop=True)
                src = ps[:].rearrange("p (h d) -> p h d", d=D)
                if mt % 2 == 0:
                    nc.scalar.copy(Vg[mt][:, :, 0:D], src)
                else:
                    nc.vector.tensor_copy(Vg[mt][:, :, 0:D], src)
                nc.scalar.activation(
                    Vg[mt][:, :, D : D + 1],
                    ident[:, 0:H].rearrange("p (a b) -> p a b", b=1),
                    Act.Identity,
                    bias=1.0,
                    scale=0.0,
                )

            qk_chain(0, 0)
            qk_chain(0, 1)

            # ---- phase 1: heads ----
            # Software-pipelined: the A^T chain for head h is interleaved with
            # head h's S^T/exp stream (lagging the exps), its last two matmuls
            # and the tail ops (recip/bcast/normalize) spill into head h+1 so
            # no engine queue ever stalls on a whole chain.
            A_tiles = {}

            def a_mm(h, j, mts):
                # A^T chain for head h, n-column half j, over m-tiles mts.
                # Each half is a [P, C] PSUM tile (1 bank) so psS can go deep.
                if (h, j) not in A_tiles:
                    A_tiles[h, j] = psA.tile([P, C], f32, tag="A", name=f"A{h}_{j}")
                for mt in mts:
                    nc.tensor.matmul(
                        A_tiles[h, j][0 : D + 1, :],
                        Vg[mt][:, h, :],
                        ET2[h % 2][mt][:, j * C : (j + 1) * C],
                        start=(mt == 0),
                        stop=(mt == NT - 1),
                    )

            def tail_a(h, j):  # after A-chain(h,j) closed: row D -> 1/r
                sl = slice(j * C, (j + 1) * C)
                nc.vector.reciprocal(rc_sb[:, sl], A_tiles[h, j][D : D + 1, :])
                nc.gpsimd.partition_broadcast(R_sb[:, sl], rc_sb[:, sl])

            def tail_b(h, j):  # normalize into mergedT, frees A psum tile
                ct, po = h // 2, (h % 2) * D
                sl = slice(j * C, (j + 1) * C)
                nc.vector.tensor_tensor(
                    mergedT[ct][po : po + D, sl],
                    A_tiles[h, j][0:D, :],
                    R_sb[:, sl],
                    op=Alu.mult,
                )

            for h in range(H):
                qt = QT[h // 2]
                kt_sb = KT[h // 2]
                po = (h % 2) * D
                for half in range(2):
                    bias_t = stream.tile(
                        [P, 4, N], bf16, tag="bias", bufs=3, name="bias_t"
                    )
                    nc.sync.dma_start(
                        out=bias_t[:],
                        in_=biasT_d[h, half * C : (half + 1) * C, :].rearrange(
                            "(g p) n -> p g n", p=P
                        ),
                    )
                    for q in range(4):
                        i = half * 4 + q
                        pe_add = i in (PE_ADD_I_QKH if 1 <= h <= 3 else PE_ADD_I)
                        S = psS.tile([P, N], f32, tag="S")
                        for j in range(2):
                            nc.tensor.matmul(
                                S[:, j * C : (j + 1) * C],
                                kt_sb[po : po + D, i * P : (i + 1) * P],
                                qt[po : po + D, j * C : (j + 1) * C],
                                start=True,
                                stop=not pe_add,
                            )
                        if pe_add:
                            for j in range(2):
                                nc.tensor.matmul(
                                    S[:, j * C : (j + 1) * C],
                                    ident_b[:],
                                    bias_t[:, q, j * C : (j + 1) * C],
                                    start=False,
                                    stop=True,
                                )
                        else:
                            nc.vector.tensor_tensor(
                                S[:], S[:], bias_t[:, q, :], op=Alu.add
                            )
                        nc.scalar.activation(
                            ET2[h % 2][i][:], S[:], Act.Exp, scale=0.125
                        )
                        # threaded work: V chains (head 0), the previous
                        # head's A-chain close + tails, later QK c-tiles
                        if h == 0:
                            if i == 0:
                                load_w_cols(wvT, wvT_d, 0, C)
                                nc.sync.dma_start(
                                    out=bv_sb[:], in_=bv_d[:].bitcast(f32r)
                                )
                            v_chain(i)
                            if i == 5:
                                load_w_cols(wqT, wqT_d, P, C)
                            elif i == 6:
                                load_w_cols(wkT, wkT_d, P, C)
                        else:
                            if i == 0:
                                a_mm(h - 1, 0, range(6, 8))  # closes j0
                            elif i == 1:
                                a_mm(h - 1, 1, range(0, 4))
                            elif i == 2:
                                a_mm(h - 1, 1, range(4, 8))  # closes j1
                            elif i == 3:
                                tail_a(h - 1, 0)
                                tail_a(h - 1, 1)
                                if h < 4:
                                    qk_chain(h, 0)
                            elif i == 4:
                                tail_b(h - 1, 0)
                                if h < 4:
                                    qk_chain(h, 1)
                            elif i == 5:
                                tail_b(h - 1, 1)
                        # this head's A-chains, lagging the exps
                        if i == 6:
                            a_mm(h, 0, range(0, 4))
                        elif i == 7:
                            a_mm(h, 0, range(4, 6))
            a_mm(H - 1, 0, range(6, 8))
            a_mm(H - 1, 1, range(0, 4))
            a_mm(H - 1, 1, range(4, 8))
            tail_a(H - 1, 0)
            tail_a(H - 1, 1)
            tail_b(H - 1, 0)
            tail_b(H - 1, 1)

            # ---- phase 2: node-similarity GEMM, upper block-triangle ----
            for i in range(NT):
                # keep the moving free dim >= 256 so f32r runs at full rate;
                # the extra columns land in the (host-mirrored) lower triangle
                coff = min(i * P, N - 2 * P)
                w = N - coff
                o_sb = stream.tile([P, N], bf16, tag="o_sb", bufs=5, name="o_sb")
                ps = psS.tile([P, N], f32, tag="S")
                # matmul outputs cannot cross a PSUM bank boundary: split the
                # free range at column 512
                chunks = [(0, min(w, C))]
                if w > C:
                    chunks.append((C, w - C))
                for c0, cw in chunks:
                    for ct in range(CT):
                        nc.tensor.matmul(
                            ps[:, c0 : c0 + cw],
                            mergedT[ct][:, i * P : (i + 1) * P],
                            mergedT[ct][:, coff + c0 : coff + c0 + cw],
                            start=(ct == 0),
                            stop=(ct == CT - 1),
                        )
                # alternate full-tile copies between Act and DVE so the DMA
                # only ever waits on one engine
                if i % 2 == 0:
                    nc.scalar.copy(o_sb[:, 0:w], ps[:, 0:w])
                else:
                    nc.vector.tensor_copy(o_sb[:, 0:w], ps[:, 0:w])
                nc.sync.dma_start(
                    out=out_d[i * P : (i + 1) * P, coff:], in_=o_sb[:, 0:w]
                )

    nc.compile()
    return nc


def _get_nc():
    if "nc" not in _CACHE:
        _CACHE["nc"] = _build_nc()
    return _CACHE["nc"]


def make_in_maps(inputs):
    x = np.asarray(inputs["x"], dtype=np.float32)
    bias = np.asarray(inputs["bias"], dtype=np.float32)
    mask = np.asarray(inputs["mask"])
    Wq = np.asarray(inputs["Wq"], dtype=np.float32)
    bq = np.asarray(inputs["bq"], dtype=np.float32)
    Wk = np.asarray(inputs["Wk"], dtype=np.float32)
    bk = np.asarray(inputs["bk"], dtype=np.float32)
    Wv = np.asarray(inputs["Wv"], dtype=np.float32)
    bv = np.asarray(inputs["bv"], dtype=np.float32)

    wqT = np.ascontiguousarray(Wq.T).astype(ml_dtypes.bfloat16)
    wkT = np.ascontiguousarray(Wk.T).astype(ml_dtypes.bfloat16)
    wvT = np.ascontiguousarray(Wv.T).astype(ml_dtypes.bfloat16)
    # bqk[p, j]: j<CT -> bq c-tiles, j>=CT -> bk c-tiles
    bqk = np.stack(
        [bq.reshape(CT, P)[j] for j in range(CT)]
        + [bk.reshape(CT, P)[j] for j in range(CT)],
        axis=1,
    ).astype(np.float32)
    bvR = np.ascontiguousarray(bv.reshape(1, C))

    in_maps = []
    for b in range(NCORES):
        mneg = (mask[b].astype(np.float32) - 1.0) * (2.0**30)  # [n, m]
        biasT = (bias[b] + mneg[None, :, :]).transpose(0, 2, 1)  # [h, m, n]
        in_maps.append(
            {
                "xT": np.ascontiguousarray(x[b].T).astype(ml_dtypes.bfloat16),
                "wqT": wqT,
                "wkT": wkT,
                "wvT": wvT,
                "bqk": np.ascontiguousarray(bqk),
                "bv": bvR,
                "biasT": np.ascontiguousarray(biasT).astype(ml_dtypes.bfloat16),
            }
        )
    return in_maps


_TRIL = None


def _mirror(o):
    """Fill the lower block-triangle from the upper (output is symmetric)."""
    global _TRIL
    if _TRIL is None:
        _TRIL = np.tril(np.ones((N, N), dtype=bool), -1)
    o = np.asarray(o).astype(np.float32)
    return np.where(_TRIL, o.T, o)


def run(inputs, trace=False, **kw):
    """Run the SPMD kernel; returns (output [8,1024,1024], BassKernelResults)."""
    from concourse.bass_utils import run_bass_kernel_spmd

    nc = _get_nc()
    in_maps = make_in_maps(inputs)
    res = run_bass_kernel_spmd(
        nc, in_maps, core_ids=list(range(NCORES)), trace=trace, **kw
    )
    out = np.stack([_mirror(res.results[i]["out"]) for i in range(NCORES)], axis=0)
    return out, res


def kernel(**inputs):
    out, _ = run(inputs)
    return out


# revision 4
# speedup vs baseline: 1.0117x; 1.0117x over previous
"""Trainium2 Bass kernel v2: multi-head attention (Graphormer-style bias+mask)
followed by a node-similarity GEMM (out = merged @ merged^T).

Sharding: pure data-parallel over batch. B=8 -> 8 NeuronCores.

Key differences vs v1:
  - S^T = K Q^T computed directly (bias supplied transposed per head from the
    host), so softmax output E^T is already in the [m-part, n-free] layout the
    A^T matmul needs -> no PE transposes.
  - mask folded into bias on the host (bf16), no identity-matmuls for mask.
  - V augmented with a ones column -> A^T matmul row 64 accumulates the
    softmax row-sums r[n] for free; normalization deferred to a single
    per-head multiply when writing mergedT (R broadcast via gpsimd).
  - final GEMM computes only the upper block-triangle (output symmetric);
    host mirrors the lower tiles.
  - batched DMAs (bias in 4-tile groups) to amortize HWDGE issue overhead.
"""

import sys

if "/opt/trn_rl_repo" not in sys.path:
    sys.path.insert(0, "/opt/trn_rl_repo")

import ml_dtypes
import numpy as np

P = 128
N = 1024
C = 512
H = 8
D = 64  # head dim
NT = N // P  # 8 row tiles
CT = C // P  # 4 channel tiles
NCORES = 8

import os

# tile indices whose bias add runs as a PE identity-matmul (rest on DVE);
# heads 1-3 carry threaded QK-projection chains so they get a lighter set
PE_ADD_I = tuple(int(c) for c in os.environ.get("KV2_PE_ADD_I", "1346"))
PE_ADD_I_QKH = tuple(int(c) for c in os.environ.get("KV2_PE_ADD_I_QKH", "346"))
WARM_MMS = int(os.environ.get("KV2_WARM_MMS", "12"))

_CACHE = {}


def _build_nc():
    import concourse.mybir as mybir
    import concourse.tile as tile
    from concourse import bacc
    from concourse.masks import make_identity

    f32 = mybir.dt.float32
    f32r = mybir.dt.float32r
    bf16 = mybir.dt.bfloat16
    Act = mybir.ActivationFunctionType
    Alu = mybir.AluOpType

    nc = bacc.Bacc("TRN2", target_bir_lowering=False, debug=False)

    # ---- DRAM parameters (per-core) ----
    xT_d = nc.dram_tensor("xT", [C, N], bf16, kind="ExternalInput")
    wqT_d = nc.dram_tensor("wqT", [C, C], bf16, kind="ExternalInput")
    wkT_d = nc.dram_tensor("wkT", [C, C], bf16, kind="ExternalInput")
    wvT_d = nc.dram_tensor("wvT", [C, C], bf16, kind="ExternalInput")
    bqk_d = nc.dram_tensor("bqk", [P, 2 * CT], f32, kind="ExternalInput")
    bv_d = nc.dram_tensor("bv", [1, C], f32, kind="ExternalInput")
    # biasT[h, m, n] = bias[h, n, m] + (mask[n, m]-1)*2^30, bf16
    biasT_d = nc.dram_tensor("biasT", [H, N, N], bf16, kind="ExternalInput")
    out_d = nc.dram_tensor("out", [N, N], bf16, kind="ExternalOutput")

    with tile.TileContext(nc) as tc:
        with (
            tc.tile_pool(name="const", bufs=1) as constp,
            tc.tile_pool(name="pers", bufs=1) as pers,
            tc.tile_pool(name="stream", bufs=1) as stream,
            tc.tile_pool(name="psS", bufs=3, space="PSUM") as psS,
            tc.tile_pool(name="psA", bufs=2, space="PSUM") as psA,
        ):
            ident = constp.tile([P, P], f32)
            make_identity(nc, ident[:])
            ident_b = constp.tile([P, P], bf16)
            nc.vector.tensor_copy(ident_b[:], ident[:])
            # ones via activation: 0*x + 1 (walrus rejects value-memsets)
            ones_row = constp.tile([1, P], f32r)
            nc.scalar.activation(
                ones_row[:], ident[0:1, :], Act.Identity, bias=1.0, scale=0.0
            )

            warm = constp.tile([P, 1], f32)
            nc.scalar.activation(warm[:], ident[:, 0:1], Act.Exp, scale=1.0)

            # Spin the PE on dummy matmuls while the input DMAs are in
            # flight: the cost model's p-state ramp needs ~3us of continuous
            # execution before matmuls run at full clock, and the PE would
            # otherwise enter the first real chain cold.
            warm_ps = psA.tile([P, C], f32, tag="A", name="warm_ps")
            for _ in range(WARM_MMS):
                nc.tensor.matmul(
                    warm_ps[:, 0:P], ident_b[:], ident_b[:], start=True, stop=True
                )

            # ---- persistent SBUF tensors ----
            QT = [pers.tile([P, N], f32r, name=f"QT{i}") for i in range(CT)]
            KT = [pers.tile([P, N], f32r, name=f"KT{i}") for i in range(CT)]
            # V_aug[mt]: [m-part, head, 65]; col 64 of each head = 1.0
            Vg = [pers.tile([P, H, D + 1], bf16, name=f"Vg{i}") for i in range(NT)]
            # double-buffered by head parity: head h+1's exps must not clobber
            # the tiles head h's spilled A-chain matmuls still read
            ET2 = [
                [pers.tile([P, N], bf16, name=f"ET{s}_{i}") for i in range(NT)]
                for s in range(2)
            ]
            mergedT = [pers.tile([P, N], f32r, name=f"mergedT{i}") for i in range(CT)]
            bqk_sb = pers.tile([P, 2 * CT], f32, name="bqk_sb")
            bv_sb = pers.tile([1, C], f32r, name="bv_sb")
            R_sb = pers.tile([D, N], f32, name="R_sb")
            rc_sb = pers.tile([1, N], f32, name="rc_sb")

            # ---- phase 0: load x^T and W^T, compute Q^T[0], K^T[0] ----
            # The V chains and the remaining Q^T/K^T c-tiles are threaded into
            # the free pipeline slots of heads 0-3 below.
            qkvp = pers
            xTt = qkvp.tile([P, CT, N], bf16, name="xT")
            wqT = qkvp.tile([P, CT, C], bf16, name="wq")
            wkT = qkvp.tile([P, CT, C], bf16, name="wk")
            wvT = qkvp.tile([P, CT, C], bf16, name="wv")

            def load_w_cols(t, dram, c0, c1):
                nc.sync.dma_start(
                    out=t[:, :, c0:c1],
                    in_=dram[:, c0:c1].rearrange("(t p) c -> p t c", p=P),
                )

            # DMA order tuned so the first QK chains + head-0 tiles start
            # as early as possible: bqk, xT g0, wq ct0, xT g1, wk ct0, then
            # bias(0,0) [emitted at head-0 top], wv+bv [head-0 slot 0],
            # remaining wq/wk columns [head-0 slots 5-6]
            nc.sync.dma_start(out=bqk_sb[:], in_=bqk_d[:])
            nc.sync.dma_start(
                out=xTt[:, 0:2, :],
                in_=xT_d[0 : 2 * P, :].rearrange("(t p) n -> p t n", p=P),
            )
            load_w_cols(wqT, wqT_d, 0, P)
            nc.sync.dma_start(
                out=xTt[:, 2:4, :],
                in_=xT_d[2 * P : 4 * P, :].rearrange("(t p) n -> p t n", p=P),
            )
            load_w_cols(wkT, wkT_d, 0, P)

            def qk_chain(ct, which):
                # one c-tile of the Q^T (which=0) / K^T (which=1) projection;
                # copy+bias alternates Act/DVE to spread phase-1 load
                w_sb, dst = ((wqT, QT), (wkT, KT))[which]
                ps = psS.tile([P, N], f32, tag="S")
                b = bqk_sb[:, which * CT + ct : which * CT + ct + 1]
                for j in range(2):
                    for kt in range(CT):
                        nc.tensor.matmul(
                            ps[:, j * C : (j + 1) * C],
                            w_sb[:, kt, ct * P : (ct + 1) * P],
                            xTt[:, kt, j * C : (j + 1) * C],
                            start=(kt == 0),
                            stop=(kt == CT - 1),
                        )
                if which == 0 or ct == 0:
                    nc.scalar.activation(dst[ct][:], ps[:], Act.Identity, bias=b)
                else:
                    nc.vector.tensor_scalar_add(dst[ct][:], ps[:], b)

            def v_chain(mt):
                # V row-tile in the augmented [head, 65] layout (col 64 = 1);
                # uses the A psum pool (1-bank tiles), copy alternates engines
                ps = psA.tile([P, C], f32, tag="A", name=f"Vps{mt}")
                for kt in range(CT):
                    nc.tensor.matmul(
                        ps[:],
                        xTt[:, kt, mt * P : (mt + 1) * P],
                        wvT[:, kt, :],
                        start=(kt == 0),
                        stop=False,
                    )
                nc.tensor.matmul(ps[:], ones_row[:], bv_sb[:], start=False, stop=True)
                src = ps[:].rearrange("p (h d) -> p h d", d=D)
                if mt % 2 == 0:
                    nc.scalar.copy(Vg[mt][:, :, 0:D], src)
                else:
                    nc.vector.tensor_copy(Vg[mt][:, :, 0:D], src)
                nc.scalar.activation(
                    Vg[mt][:, :, D : D + 1],
                    ident[:, 0:H].rearrange("p (a b) -> p a b", b=1),
                    Act.Identity,
                    bias=1.0,
                    scale=0.0,
                )

            qk_chain(0, 0)
            qk_chain(0, 1)

            # ---- phase 1: heads ----
            # Software-pipelined: the A^T chain for head h is interleaved with
            # head h's S^T/exp stream (lagging the exps), its last two matmuls
            # and the tail ops (recip/bcast/normalize) spill into head h+1 so
            # no engine queue ever stalls on a whole chain.
            A_tiles = {}

            def a_mm(h, j, mts):
                # A^T chain for head h, n-column half j, over m-tiles mts.
                # Each half is a [P, C] PSUM tile (1 bank) so psS can go deep.
                if (h, j) not in A_tiles:
                    A_tiles[h, j] = psA.tile([P, C], f32, tag="A", name=f"A{h}_{j}")
                for mt in mts:
                    nc.tensor.matmul(
                        A_tiles[h, j][0 : D + 1, :],
                        Vg[mt][:, h, :],
                        ET2[h % 2][mt][:, j * C : (j + 1) * C],
                        start=(mt == 0),
                        stop=(mt == NT - 1),
                    )

            def tail_a(h, j):  # after A-chain(h,j) closed: row D -> 1/r
                sl = slice(j * C, (j + 1) * C)
                nc.vector.reciprocal(rc_sb[:, sl], A_tiles[h, j][D : D + 1, :])
                nc.gpsimd.partition_broadcast(R_sb[:, sl], rc_sb[:, sl])

            def tail_b(h, j):  # normalize into mergedT, frees A psum tile
                ct, po = h // 2, (h % 2) * D
                sl = slice(j * C, (j + 1) * C)
                nc.vector.tensor_tensor(
                    mergedT[ct][po : po + D, sl],
                    A_tiles[h, j][0:D, :],
                    R_sb[:, sl],
                    op=Alu.mult,
                )

            for h in range(H):
                qt = QT[h // 2]
                kt_sb = KT[h // 2]
                po = (h % 2) * D
                for half in range(2):
                    bias_t = stream.tile(
                        [P, 4, N], bf16, tag="bias", bufs=3, name="bias_t"
                    )
                    nc.sync.dma_start(
                        out=bias_t[:],
                        in_=biasT_d[h, half * C : (half + 1) * C, :].rearrange(
                            "(g p) n -> p g n", p=P
                        ),
                    )
                    for q in range(4):
                        i = half * 4 + q
                        pe_add = i in (PE_ADD_I_QKH if 1 <= h <= 3 else PE_ADD_I)
                        S = psS.tile([P, N], f32, tag="S")
                        for j in range(2):
                            nc.tensor.matmul(
                                S[:, j * C : (j + 1) * C],
                                kt_sb[po : po + D, i * P : (i + 1) * P],
                                qt[po : po + D, j * C : (j + 1) * C],
                                start=True,
                                stop=not pe_add,
                            )
                        if pe_add:
                            for j in range(2):
                                nc.tensor.matmul(
                                    S[:, j * C : (j + 1) * C],
                                    ident_b[:],
                                    bias_t[:, q, j * C : (j + 1) * C],
                                    start=False,
                                    stop=True,
                                )
                        else:
                            nc.vector.tensor_tensor(
                                S[:], S[:], bias_t[:, q, :], op=Alu.add
                            )
                        nc.scalar.activation(
                            ET2[h % 2][i][:], S[:], Act.Exp, scale=0.125
                        )
                        # threaded work: V chains (head 0), the previous
                        # head's A-chain close + tails, later QK c-tiles
                        if h == 0:
                            if i == 0:
                                load_w_cols(wvT, wvT_d, 0, C)
                                nc.sync.dma_start(
                                    out=bv_sb[:], in_=bv_d[:].bitcast(f32r)
                                )
                            v_chain(i)
                            if i == 5:
                                load_w_cols(wqT, wqT_d, P, C)
                            elif i == 6:
                                load_w_cols(wkT, wkT_d, P, C)
                        else:
                            if i == 0:
                                a_mm(h - 1, 0, range(6, 8))  # closes j0
                            elif i == 1:
                                a_mm(h - 1, 1, range(0, 4))
                            elif i == 2:
                                a_mm(h - 1, 1, range(4, 8))  # closes j1
                            elif i == 3:
                                tail_a(h - 1, 0)
                                tail_a(h - 1, 1)
                                if h < 4:
                                    qk_chain(h, 0)
                            elif i == 4:
                                tail_b(h - 1, 0)
                                if h < 4:
                                    qk_chain(h, 1)
                            elif i == 5:
                                tail_b(h - 1, 1)
                        # this head's A-chains, lagging the exps
                        if i == 6:
                            a_mm(h, 0, range(0, 4))
                        elif i == 7:
                            a_mm(h, 0, range(4, 6))
            a_mm(H - 1, 0, range(6, 8))
            tail_a(H - 1, 0)
            a_mm(H - 1, 1, range(0, 4))
            a_mm(H - 1, 1, range(4, 8))
            tail_a(H - 1, 1)
            tail_b(H - 1, 0)
            tail_b(H - 1, 1)

            # ---- phase 2: node-similarity GEMM, upper block-triangle ----
            o67 = None
            for i in range(NT):
                # keep the moving free dim >= 256 so f32r runs at full rate;
                # the extra columns land in the (host-mirrored) lower triangle
                coff = min(i * P, N - 2 * P)
                w = N - coff
                if i < 6:
                    o_sb = stream.tile([P, N], bf16, tag="o_sb", bufs=5, name="o_sb")
                else:
                    # tiles 6 and 7 share one buffer and a single merged DMA
                    if o67 is None:
                        o67 = stream.tile([P, 2, 2 * P], bf16, name="o67")
                    o_sb = o67[:, i - 6, :]
                ps = psS.tile([P, N], f32, tag="S")
                # matmul outputs cannot cross a PSUM bank boundary: split the
                # free range at column 512
                chunks = [(0, min(w, C))]
                if w > C:
                    chunks.append((C, w - C))
                for c0, cw in chunks:
                    for ct in range(CT):
                        nc.tensor.matmul(
                            ps[:, c0 : c0 + cw],
                            mergedT[ct][:, i * P : (i + 1) * P],
                            mergedT[ct][:, coff + c0 : coff + c0 + cw],
                            start=(ct == 0),
                            stop=(ct == CT - 1),
                        )
                # alternate full-tile copies between Act and DVE so the DMA
                # only ever waits on one engine; split the final tiles across
                # both engines to shorten the drain tail
                if i >= 6:
                    wh = w // 2
                    nc.scalar.copy(o_sb[:, 0:wh], ps[:, 0:wh])
                    nc.vector.tensor_copy(o_sb[:, wh:w], ps[:, wh:w])
                    if i == 7:
                        nc.sync.dma_start(
                            out=out_d[6 * P :, 6 * P :].rearrange(
                                "(g p) n -> p g n", p=P
                            ),
                            in_=o67[:],
                        )
                elif i % 2 == 0:
                    nc.scalar.copy(o_sb[:, 0:w], ps[:, 0:w])
                    nc.sync.dma_start(
                        out=out_d[i * P : (i + 1) * P, coff:], in_=o_sb[:, 0:w]
                    )
                else:
                    nc.vector.tensor_copy(o_sb[:, 0:w], ps[:, 0:w])
                    nc.sync.dma_start(
                        out=out_d[i * P : (i + 1) * P, coff:], in_=o_sb[:, 0:w]
                    )

    nc.compile()
    return nc


def _get_nc():
    if "nc" not in _CACHE:
        _CACHE["nc"] = _build_nc()
    return _CACHE["nc"]


def make_in_maps(inputs):
    x = np.asarray(inputs["x"], dtype=np.float32)
    bias = np.asarray(inputs["bias"], dtype=np.float32)
    mask = np.asarray(inputs["mask"])
    Wq = np.asarray(inputs["Wq"], dtype=np.float32)
    bq = np.asarray(inputs["bq"], dtype=np.float32)
    Wk = np.asarray(inputs["Wk"], dtype=np.float32)
    bk = np.asarray(inputs["bk"], dtype=np.float32)
    Wv = np.asarray(inputs["Wv"], dtype=np.float32)
    bv = np.asarray(inputs["bv"], dtype=np.float32)

    wqT = np.ascontiguousarray(Wq.T).astype(ml_dtypes.bfloat16)
    wkT = np.ascontiguousarray(Wk.T).astype(ml_dtypes.bfloat16)
    wvT = np.ascontiguousarray(Wv.T).astype(ml_dtypes.bfloat16)
    # bqk[p, j]: j<CT -> bq c-tiles, j>=CT -> bk c-tiles
    bqk = np.stack(
        [bq.reshape(CT, P)[j] for j in range(CT)]
        + [bk.reshape(CT, P)[j] for j in range(CT)],
        axis=1,
    ).astype(np.float32)
    bvR = np.ascontiguousarray(bv.reshape(1, C))

    in_maps = []
    for b in range(NCORES):
        mneg = (mask[b].astype(np.float32) - 1.0) * (2.0**30)  # [n, m]
        biasT = (bias[b] + mneg[None, :, :]).transpose(0, 2, 1)  # [h, m, n]
        in_maps.append(
            {
                "xT": np.ascontiguousarray(x[b].T).astype(ml_dtypes.bfloat16),
                "wqT": wqT,
                "wkT": wkT,
                "wvT": wvT,
                "bqk": np.ascontiguousarray(bqk),
                "bv": bvR,
                "biasT": np.ascontiguousarray(biasT).astype(ml_dtypes.bfloat16),
            }
        )
    return in_maps


_TRIL = None


def _mirror(o):
    """Fill the lower block-triangle from the upper (output is symmetric)."""
    global _TRIL
    if _TRIL is None:
        _TRIL = np.tril(np.ones((N, N), dtype=bool), -1)
    o = np.asarray(o).astype(np.float32)
    return np.where(_TRIL, o.T, o)


def run(inputs, trace=False, **kw):
    """Run the SPMD kernel; returns (output [8,1024,1024], BassKernelResults)."""
    from concourse.bass_utils import run_bass_kernel_spmd

    nc = _get_nc()
    in_maps = make_in_maps(inputs)
    res = run_bass_kernel_spmd(
        nc, in_maps, core_ids=list(range(NCORES)), trace=trace, **kw
    )
    out = np.stack([_mirror(res.results[i]["out"]) for i in range(NCORES)], axis=0)
    return out, res


def kernel(**inputs):
    out, _ = run(inputs)
    return out
